# revision 13
# baseline (speedup 1.0000x reference)
"""Trainium2 Bass kernel for a dense transformer block (B=4, T=2048, D=1024, H=16).

Sharding: 8 cores = 4 batches x 2 head-halves.  Each core computes LN1
(folded into weights host-side), Q/K/V for its 8 heads over all 2048
tokens, causal attention in feature-major layout (denominator via a
ones-column appended to V), then a row-parallel Wo partial that is
pairwise ReduceScattered so that each core ends up with a 1024-token
half for LN2 + MLP.

v2 changes vs baseline:
- bf16 operands on every matmul path (same PE rate, half the DMA/SBUF,
  2x DVE); fp32 only on the residual stream (x2) and output.
- qkv/wo weights resident in SBUF, loaded once; w1/w2 streamed bf16.
- x = x*rstd pre-scaled once per tile (folds LN into the operand),
  removing per-block rstd multiplies and the token-major rstd bounce.
- x2 kept in SBUF (no DRAM round-trip); ReduceScatter in bf16.
- LN rstd via exp(-0.5*ln(var+eps)) so the attention region never
  swaps the ACT function table away from the exp set.
- MLP emitted after the last ReduceScatter is issued, hiding it.
"""

import os
import sys

for _p in ("/opt/trn_rl_repo", "/root/.axon_site/_ro/trn_rl_repo"):
    if os.path.isdir(_p) and _p not in sys.path:
        sys.path.append(_p)

import numpy as np

import concourse.bass as bass
import concourse.tile as tile
from concourse import bacc, mybir
from concourse.bass_utils import run_bass_kernel_spmd

AF = mybir.ActivationFunctionType
ALU = mybir.AluOpType
FP32 = mybir.dt.float32
FP32R = mybir.dt.float32r
BF16 = mybir.dt.bfloat16

B, T, D, H = 4, 2048, 1024, 16
HD = D // H          # 64
DFF = 4 * D          # 4096
P = 128
DK = D // P          # 8   D k-tiles
NT = T // 512        # 4   512-token tiles
HC = H // 2          # 8   local heads per core
DQ = HC * HD         # 512 local qkv width
NOT = DQ // P        # 4   local qkv feature tiles (head pairs)
FFT = DFF // P       # 32  DFF tiles
TOWN = T // 2        # 1024 own tokens after ReduceScatter
NTO = TOWN // 512    # 2
EPS = 1e-5
SCALE = 1.0 / 8.0    # 1/sqrt(HD)


def build_program(debug=False, sim_mode=False):
    nc = bacc.Bacc("TRN2", target_bir_lowering=False, debug=False)

    # ---- DRAM I/O ----
    xT = nc.dram_tensor("xT", [D, T], BF16, kind="ExternalInput")
    xTo = nc.dram_tensor("xTo", [D, TOWN], FP32, kind="ExternalInput")
    wqk = nc.dram_tensor("wqk", [2, NOT, DK, P, P], BF16, kind="ExternalInput")
    wv = nc.dram_tensor("wv", [DK, P, DQ], BF16, kind="ExternalInput")
    wo = nc.dram_tensor("wo", [DQ, D], BF16, kind="ExternalInput")
    w1 = nc.dram_tensor("w1", [FFT, DK, P, P], BF16, kind="ExternalInput")
    w2 = nc.dram_tensor("w2", [DK, FFT, P, P], BF16, kind="ExternalInput")
    cqk = nc.dram_tensor("cqk", [P, 2 * NOT], FP32, kind="ExternalInput")
    cvb = nc.dram_tensor("cvb", [P, DQ], BF16, kind="ExternalInput")
    bo = nc.dram_tensor("bo", [P, DK], FP32, kind="ExternalInput")
    c1 = nc.dram_tensor("c1", [P, FFT], FP32, kind="ExternalInput")
    b2 = nc.dram_tensor("b2", [P, DK], FP32, kind="ExternalInput")
    masks = nc.dram_tensor("masks", [P, 896], BF16, kind="ExternalInput")
    out = nc.dram_tensor("out", [DK, P, TOWN], FP32, kind="ExternalOutput")

    xT_r = xT.rearrange("(k p) t -> p k t", p=P)
    xTo_r = xTo.rearrange("(k p) t -> p k t", p=P)
    wo_r = wo.rearrange("(pt p) o -> p pt o", p=P)
    out_r = out.rearrange("k p t -> p k t")

    with tile.TileContext(nc) as tc:
        with (
            tc.tile_pool(name="small", bufs=1) as small,
            tc.tile_pool(name="psum", bufs=1, space="PSUM") as psum,
            tc.tile_pool(name="dram", bufs=1, space="DRAM") as dram,
        ):
            # ---- persistent small tiles / resident weights ----
            ones_bf = small.tile([P, 1], BF16)
            nc.vector.memset(ones_bf, 1.0)
            ones_r = small.tile([P, 1], FP32R)
            nc.vector.memset(ones_r.bitcast(FP32), 1.0)
            cqk_sb = small.tile([P, 2 * NOT], FP32)
            nc.sync.dma_start(cqk_sb, cqk[:, :])
            cvb_sb = small.tile([P, DQ], BF16)
            nc.sync.dma_start(cvb_sb, cvb[:, :])
            bo_sb = small.tile([P, DK], FP32)
            nc.sync.dma_start(bo_sb, bo[:, :])
            c1_sb = small.tile([P, FFT], FP32)
            nc.sync.dma_start(c1_sb, c1[:, :])
            b2_sb = small.tile([P, DK], FP32)
            nc.sync.dma_start(b2_sb, b2[:, :])
            eps_sb = small.tile([1, 1], FP32)
            nc.vector.memset(eps_sb, EPS)

            # resident weights (bf16, loaded once)
            wqk_sb = small.tile([P, 2, NOT, DK, P], BF16)
            for proj in range(2):
                nc.gpsimd.dma_start(
                    wqk_sb[:, proj], wqk[proj].rearrange("b k p m -> p b k m"))
            wv_sb = small.tile([P, DK, DQ], BF16)
            nc.gpsimd.dma_start(wv_sb, wv.rearrange("k p m -> p k m"))
            wo_sb = small.tile([P, NOT, D], BF16)
            nc.gpsimd.dma_start(wo_sb, wo_r)

            # persistent activations
            x2_sb = small.tile([P, DK, TOWN], FP32R)    # residual stream
            rsb2_sb = small.tile([P, TOWN], BF16)

            # DRAM scratch for the pairwise ReduceScatter (bf16)
            rs_in = [dram.tile([2, DK, P, 512], BF16, name=f"rsin{i}", tag=f"rsin{i}")
                     for i in range(2)]
            rs_out = [dram.tile([DK, P, 512], BF16, name=f"rsout{i}", tag=f"rsout{i}")
                      for i in range(2)]

            # ==== phases A-E: software-pipelined over 512-token tiles ====
            # iter tt: A/B(tt) stats+projections; C/D(tt-1) attention+Wo;
            # ReduceScatter + x2 + LN2 for token-half (tt-3).
            with (
                tc.tile_pool(name="kv", bufs=1) as kvp,
                tc.tile_pool(name="abc", bufs=1) as abc,
            ):
                kT_sb = kvp.tile([P, NOT, T], BF16)     # [64*(h%2)+d, h//2, t]
                v_sb = kvp.tile([P, HC, T // P, HD + 1], BF16)
                mask_sb = kvp.tile([P, 896], BF16)      # extended causal mask
                nc.sync.dma_start(mask_sb, masks[:, :])
                for h in range(HC):
                    nc.vector.memset(v_sb[:, h, :, HD:HD + 1], 1.0)

                qcur_t = [None] * NT
                for tt in range(NT + 1):
                    if tt < NT:
                        ts5 = slice(tt * 512, (tt + 1) * 512)
                        # ---- A: load x tile, LN1 stats, prescale ----
                        xt_t = abc.tile([P, DK, 512], BF16, tag="xt", bufs=2)
                        nc.sync.dma_start(xt_t, xT_r[:, :, ts5])
                        s_ps = psum.tile([1, 512], FP32, tag="st", bufs=2)
                        q_ps = psum.tile([1, 512], FP32, tag="st", bufs=2)
                        for kt in range(DK):
                            xsq = abc.tile([P, 512], BF16, tag="xsq", bufs=2)
                            nc.vector.tensor_mul(xsq, xt_t[:, kt, :], xt_t[:, kt, :])
                            nc.tensor.matmul(s_ps, ones_bf, xt_t[:, kt, :],
                                             start=(kt == 0), stop=(kt == DK - 1))
                            nc.tensor.matmul(q_ps, ones_bf, xsq,
                                             start=(kt == 0), stop=(kt == DK - 1))
                        # rows: mu, var; rstd = exp(-0.5*ln(var+eps))
                        mu = abc.tile([1, 512], FP32, tag="murow", bufs=2)
                        va = abc.tile([1, 512], FP32, tag="varow", bufs=2)
                        nc.vector.tensor_scalar(mu, s_ps, 1.0 / D, None, ALU.mult)
                        nc.vector.tensor_scalar(va, q_ps, 1.0 / D, None, ALU.mult)
                        nc.gpsimd.tensor_mul(mu, mu, mu)
                        nc.gpsimd.tensor_sub(va, va, mu)
                        lnv = abc.tile([1, 512], FP32, tag="lnrow", bufs=2)
                        nc.scalar.activation(lnv, va, AF.Ln, bias=eps_sb)
                        rstd_row = abc.tile([1, 512], BF16, tag="rsrow", bufs=2)
                        nc.scalar.activation(rstd_row, lnv, AF.Exp, scale=-0.5)
                        rsb = abc.tile([P, 512], BF16, tag="rsb", bufs=2)
                        nc.gpsimd.partition_broadcast(rsb, rstd_row)
                        # xh = x * rstd  (bf16, the matmul operand everywhere)
                        xh_t = abc.tile([P, DK, 512], BF16, tag="xh", bufs=2)
                        for kt in range(DK):
                            nc.vector.tensor_mul(xh_t[:, kt, :], xt_t[:, kt, :], rsb)

                        # ---- B: q/k projections (feature-major out) ----
                        qcur = abc.tile([P, NOT, 512], BF16, tag="qcur", bufs=2)
                        qcur_t[tt] = qcur
                        for proj in range(2):  # 0=q, 1=k
                            for ot in range(NOT):
                                pp = psum.tile([P, 512], FP32, tag="mm", bufs=2)
                                for kt in range(DK):
                                    nc.tensor.matmul(
                                        pp, wqk_sb[:, proj, ot, kt, :],
                                        xh_t[:, kt, :],
                                        start=(kt == 0), stop=(kt == DK - 1))
                                dest = (qcur[:, ot, :] if proj == 0
                                        else kT_sb[:, ot, ts5])
                                nc.vector.tensor_scalar(
                                    dest, pp,
                                    cqk_sb[:, proj * NOT + ot:proj * NOT + ot + 1],
                                    None, ALU.add)
                        # ---- B: v projection (token-major out) ----
                        for st in range(4):
                            pp = psum.tile([P, 512], FP32, tag="mm", bufs=2)
                            for kt in range(DK):
                                nc.tensor.matmul(
                                    pp, xh_t[:, kt, st * P:(st + 1) * P],
                                    wv_sb[:, kt, :], start=(kt == 0),
                                    stop=(kt == DK - 1))
                            nc.vector.tensor_tensor(
                                v_sb[:, :, tt * 4 + st, 0:HD],
                                pp.rearrange("p (h e) -> p h e", h=HC),
                                cvb_sb.rearrange("p (h e) -> p h e", h=HC), ALU.add)

                    if tt >= 1:
                        # ---- C: attention for q-tile qt = tt-1 ----
                        qt = tt - 1
                        qv = qcur_t[qt]
                        nkt = 4 * qt + 4
                        ysb = abc.tile([P, NOT, 512], BF16, tag="ysb", bufs=1)
                        for pt in range(NOT):
                            y_ps = [psum.tile([HD + 1, 512], FP32, name=f"yps{hb}",
                                              tag="y", bufs=2) for hb in range(2)]
                            for kt in range(nkt):
                                jband = kt - 4 * qt
                                # queries below 128*jband cannot attend to this
                                # key block: restrict all work to [q0, 512).
                                q0 = P * jband if jband > 0 else 0
                                qs = slice(q0, 512)
                                pexp = []
                                for hb in range(2):
                                    hsl = slice(hb * HD, (hb + 1) * HD)
                                    s_ps2 = psum.tile([P, 512], FP32, tag="s", bufs=2)
                                    nc.tensor.matmul(
                                        s_ps2[:, qs],
                                        kT_sb[hsl, pt, kt * P:(kt + 1) * P],
                                        qv[hsl, pt, qs], start=True, stop=True)
                                    pe = abc.tile([P, 512], BF16, tag="pexp", bufs=3)
                                    nc.scalar.activation(pe[:, qs], s_ps2[:, qs],
                                                         AF.Exp, scale=SCALE)
                                    if jband >= 0:
                                        moff = 384 - P * jband
                                        nc.vector.tensor_mul(
                                            pe[:, qs], pe[:, qs],
                                            mask_sb[:, moff + q0:moff + 512])
                                    pexp.append(pe)
                                for hb in range(2):
                                    nc.tensor.matmul(
                                        y_ps[hb][:, qs],
                                        v_sb[:, 2 * pt + hb, kt, :], pexp[hb][:, qs],
                                        start=(kt == 0), stop=(kt == nkt - 1))
                                # PE idle-filler: keeps the HAM clock warm
                                # through the ACT-bound attention stretch.
                                nc.tensor.ldweights(wqk_sb[:, 0, 0, 0, :])
                                nc.tensor.ldweights(wqk_sb[:, 1, 0, 0, :])
                            for hb in range(2):
                                # denominator: reciprocal on its own lane, DMA
                                # the row down to lane 0, gpsimd-broadcast.
                                den = abc.tile([HD + 1, 512], FP32, tag="den", bufs=2)
                                nc.vector.reciprocal(den[HD:HD + 1, :],
                                                     y_ps[hb][HD:HD + 1, :])
                                rec = abc.tile([1, 512], FP32, tag="rec", bufs=2)
                                nc.gpsimd.dma_start(rec, den[HD:HD + 1, :])
                                rb = abc.tile([HD, 512], FP32, tag="rb", bufs=2)
                                nc.gpsimd.partition_broadcast(rb, rec)
                                if hb == 0:
                                    nc.vector.tensor_mul(ysb[0:HD, pt, :],
                                                         y_ps[hb][0:HD, :], rb)
                                else:
                                    yst = abc.tile([HD, 512], BF16, tag="yst", bufs=2)
                                    nc.vector.tensor_mul(yst, y_ps[hb][0:HD, :], rb)
                                    nc.gpsimd.dma_start(ysb[HD:2 * HD, pt, :], yst)
                        # ---- D: Wo partials for q-tile qt ----
                        for ot in range(DK):
                            pp = psum.tile([P, 512], FP32, tag="mm", bufs=2)
                            for pt in range(NOT):
                                nc.tensor.matmul(
                                    pp, wo_sb[:, pt, ot * P:(ot + 1) * P],
                                    ysb[:, pt, :],
                                    start=(pt == 0), stop=(pt == NOT - 1))
                            ast = abc.tile([P, 512], BF16, tag="ast", bufs=2)
                            nc.vector.tensor_copy(ast, pp)
                            nc.sync.dma_start(rs_in[qt % 2][qt // 2, ot], ast)
                        if qt >= 2:
                            # ---- ReduceScatter + x2 + LN2 for half i ----
                            i = qt - 2
                            io5 = slice(i * 512, (i + 1) * 512)
                            if sim_mode:
                                nc.sync.dma_start(rs_out[i][:, :, :], rs_in[i][0])
                            else:
                                nc.gpsimd.collective_compute(
                                    "ReduceScatter", ALU.add,
                                    replica_groups=[[0, 1], [2, 3], [4, 5], [6, 7]],
                                    ins=[rs_in[i].opt()], outs=[rs_out[i].opt()])
                            s2_ps = psum.tile([1, 512], FP32, tag="st", bufs=2)
                            q2_ps = psum.tile([1, 512], FP32, tag="st", bufs=2)
                            for kt in range(DK):
                                att = abc.tile([P, 512], BF16, tag="att", bufs=2)
                                nc.sync.dma_start(att, rs_out[i][kt])
                                xo_t = abc.tile([P, 512], FP32, tag="xo", bufs=2)
                                nc.sync.dma_start(xo_t, xTo_r[:, kt, io5])
                                nc.vector.scalar_tensor_tensor(
                                    x2_sb[:, kt, io5], att, bo_sb[:, kt:kt + 1],
                                    xo_t, ALU.add, ALU.add)
                                xsq2 = abc.tile([P, 512], FP32R, tag="xsq2", bufs=2)
                                nc.vector.tensor_mul(xsq2, x2_sb[:, kt, io5],
                                                     x2_sb[:, kt, io5])
                                nc.tensor.matmul(s2_ps, ones_r, x2_sb[:, kt, io5],
                                                 start=(kt == 0), stop=(kt == DK - 1))
                                nc.tensor.matmul(q2_ps, ones_r, xsq2,
                                                 start=(kt == 0), stop=(kt == DK - 1))
                            mu2 = abc.tile([1, 512], FP32, tag="murow", bufs=2)
                            va2 = abc.tile([1, 512], FP32, tag="varow", bufs=2)
                            nc.vector.tensor_scalar(mu2, s2_ps, 1.0 / D, None, ALU.mult)
                            nc.vector.tensor_scalar(va2, q2_ps, 1.0 / D, None, ALU.mult)
                            nc.gpsimd.tensor_mul(mu2, mu2, mu2)
                            nc.gpsimd.tensor_sub(va2, va2, mu2)
                            lnv2 = abc.tile([1, 512], FP32, tag="lnrow", bufs=2)
                            nc.scalar.activation(lnv2, va2, AF.Ln, bias=eps_sb)
                            rstd2_row = abc.tile([1, 512], BF16, tag="rsrow", bufs=2)
                            nc.scalar.activation(rstd2_row, lnv2, AF.Exp, scale=-0.5)
                            nc.gpsimd.partition_broadcast(rsb2_sb[:, io5], rstd2_row)

            # =========== phases F/G: MLP over 512-token tiles ==========
            with tc.tile_pool(name="fg", bufs=1) as fg:
                for tt in range(NTO):
                    ts5 = slice(tt * 512, (tt + 1) * 512)
                    xh2_t = fg.tile([P, DK, 512], BF16, tag="xh2", bufs=1)
                    for kt in range(DK):
                        nc.vector.tensor_mul(xh2_t[:, kt, :], x2_sb[:, kt, ts5],
                                             rsb2_sb[:, ts5])
                    m_sb = fg.tile([P, FFT, 512], BF16, tag="m", bufs=1)
                    for fft in range(FFT):
                        w1b = fg.tile([P, DK, P], BF16, tag="w1b", bufs=3)
                        nc.gpsimd.dma_start(w1b, w1[fft].rearrange("k p m -> p k m"))
                        pp = psum.tile([P, 512], FP32, tag="mm", bufs=2)
                        for kt in range(DK):
                            nc.tensor.matmul(pp, w1b[:, kt, :], xh2_t[:, kt, :],
                                             start=(kt == 0),
                                             stop=(kt == DK - 1))
                        nc.scalar.activation(m_sb[:, fft, :], pp, AF.Gelu,
                                             bias=c1_sb[:, fft:fft + 1])
                    for ot in range(DK):
                        pp = psum.tile([P, 512], FP32, tag="mm", bufs=2)
                        for half in range(2):
                            w2b = fg.tile([P, FFT // 2, P], BF16, tag="w2b", bufs=2)
                            nc.gpsimd.dma_start(
                                w2b, w2[ot, half * (FFT // 2):(half + 1) * (FFT // 2)]
                                .rearrange("k p m -> p k m"))
                            for kk in range(FFT // 2):
                                kt = half * (FFT // 2) + kk
                                nc.tensor.matmul(pp, w2b[:, kk, :], m_sb[:, kt, :],
                                                 start=(kt == 0), stop=(kt == FFT - 1))
                        ost = fg.tile([P, 512], FP32, tag="ost", bufs=2)
                        nc.vector.scalar_tensor_tensor(
                            ost, pp, b2_sb[:, ot:ot + 1], x2_sb[:, ot, ts5],
                            ALU.add, ALU.add)
                        nc.sync.dma_start(out_r[:, ot, ts5], ost)

    nc.compile()
    return nc


_NC_CACHE = None


def _get_nc():
    global _NC_CACHE
    if _NC_CACHE is None:
        _NC_CACHE = build_program(debug=bool(int(os.environ.get("KERNEL_DEBUG", "0"))))
    return _NC_CACHE


def prep_in_maps(x, ln1_g, ln1_b, ln2_g, ln2_b, Wq, bq, Wk, bk, Wv, bv,
                 Wo, bo, W1, b1, W2, b2):
    from ml_dtypes import bfloat16
    f32 = np.float32
    x = np.asarray(x, f32)
    ln1_g, ln1_b = np.asarray(ln1_g, f32), np.asarray(ln1_b, f32)
    ln2_g, ln2_b = np.asarray(ln2_g, f32), np.asarray(ln2_b, f32)
    Wq, Wk, Wv, Wo = (np.asarray(a, f32) for a in (Wq, Wk, Wv, Wo))
    W1, W2 = np.asarray(W1, f32), np.asarray(W2, f32)
    bq, bk, bv, bo_, b1, b2_ = (np.asarray(a, f32) for a in (bq, bk, bv, bo, b1, b2))

    # fold LN gain AND the mean subtraction (a rank-1 correction) into W:
    # (x - mu) * g @ W  =  x @ (g*W - colsum(g*W)/D)
    Wqg = ln1_g[:, None] * Wq
    Wkg = ln1_g[:, None] * Wk
    Wvg = ln1_g[:, None] * Wv
    Wqg = Wqg - Wqg.sum(0, keepdims=True) / D
    Wkg = Wkg - Wkg.sum(0, keepdims=True) / D
    Wvg = Wvg - Wvg.sum(0, keepdims=True) / D
    cq_full = ln1_b @ Wq + bq
    ck_full = ln1_b @ Wk + bk
    cv_full = ln1_b @ Wv + bv
    W1g = ln2_g[:, None] * W1
    W1g = W1g - W1g.sum(0, keepdims=True) / D
    c1_full = ln2_b @ W1 + b1

    w1_t = np.ascontiguousarray(
        W1g.reshape(DK, P, FFT, P).transpose(2, 0, 1, 3)).astype(bfloat16)
    w2_t = np.ascontiguousarray(
        W2.reshape(FFT, P, DK, P).transpose(2, 0, 1, 3)).astype(bfloat16)
    c1_t = np.ascontiguousarray(c1_full.reshape(FFT, P).T)      # [P,FFT]
    b2_t = np.ascontiguousarray(b2_.reshape(DK, P).T)           # [P,DK]
    bo_t = np.ascontiguousarray(bo_.reshape(DK, P).T)           # [P,DK]

    kk = np.arange(P)[:, None]
    cc = np.arange(896)[None, :]
    mk = (kk + 384 <= cc).astype(bfloat16)

    in_maps = []
    for c in range(8):
        b_idx, hh = c // 2, c % 2
        sl = slice(DQ * hh, DQ * hh + DQ)
        xT_c = np.ascontiguousarray(x[b_idx].T)
        wq_c, wk_c = Wqg[:, sl], Wkg[:, sl]
        wqk_t = np.stack([
            np.ascontiguousarray(w.reshape(DK, P, NOT, P).transpose(2, 0, 1, 3))
            for w in (wq_c, wk_c)]).astype(bfloat16)             # [2,NOT,DK,P,P]
        cq_t = cq_full[sl].reshape(NOT, P).T                     # [P,NOT]
        ck_t = ck_full[sl].reshape(NOT, P).T
        in_maps.append({
            "xT": xT_c.astype(bfloat16),
            "xTo": np.ascontiguousarray(xT_c[:, hh * TOWN:(hh + 1) * TOWN]),
            "wqk": wqk_t,
            "wv": np.ascontiguousarray(Wvg[:, sl].reshape(DK, P, DQ)).astype(bfloat16),
            "wo": np.ascontiguousarray(Wo[sl, :]).astype(bfloat16),
            "w1": w1_t,
            "w2": w2_t,
            "cqk": np.ascontiguousarray(np.concatenate([cq_t, ck_t], axis=1)),
            "cvb": np.broadcast_to(cv_full[sl][None, :], (P, DQ)).astype(bfloat16),
            "bo": bo_t,
            "c1": c1_t,
            "b2": b2_t,
            "masks": mk,
        })
    return in_maps


def assemble_output(results):
    out = np.empty((B, T, D), np.float32)
    for c in range(8):
        b_idx, hh = c // 2, c % 2
        o = results[c]["out"].reshape(D, TOWN)
        out[b_idx, hh * TOWN:(hh + 1) * TOWN, :] = o.T
    return out


def kernel(**inputs):
    nc = _get_nc()
    in_maps = prep_in_maps(**inputs)
    res = run_bass_kernel_spmd(nc, in_maps, list(range(8)))
    return assemble_output(res.results)


# revision 14
# speedup vs baseline: 1.7873x; 1.7873x over previous
"""Trainium2 Bass kernel for a dense transformer block (B=4, T=2048, D=1024, H=16).

Sharding: 8 cores = 4 batches x 2 head-halves.  Each core computes LN1
(folded into weights host-side), Q/K/V for its 8 heads over all 2048
tokens, causal attention in feature-major layout (denominator via a
ones-column appended to V), then a row-parallel Wo partial that is
pairwise ReduceScattered so that each core ends up with a 1024-token
half for LN2 + MLP.

v2 changes vs baseline:
- bf16 operands on every matmul path (same PE rate, half the DMA/SBUF,
  2x DVE); fp32 only on the residual stream (x2) and output.
- qkv/wo weights resident in SBUF, loaded once; w1/w2 streamed bf16.
- x = x*rstd pre-scaled once per tile (folds LN into the operand),
  removing per-block rstd multiplies and the token-major rstd bounce.
- x2 kept in SBUF (no DRAM round-trip); ReduceScatter in bf16.
- LN rstd via exp(-0.5*ln(var+eps)) so the attention region never
  swaps the ACT function table away from the exp set.
- MLP emitted after the last ReduceScatter is issued, hiding it.
"""

import os
import sys

for _p in ("/opt/trn_rl_repo", "/root/.axon_site/_ro/trn_rl_repo"):
    if os.path.isdir(_p) and _p not in sys.path:
        sys.path.append(_p)

import numpy as np

import concourse.bass as bass
import concourse.tile as tile
from concourse import bacc, mybir
from concourse.bass_utils import run_bass_kernel_spmd

AF = mybir.ActivationFunctionType
ALU = mybir.AluOpType
FP32 = mybir.dt.float32
FP32R = mybir.dt.float32r
BF16 = mybir.dt.bfloat16

B, T, D, H = 4, 2048, 1024, 16
HD = D // H          # 64
DFF = 4 * D          # 4096
P = 128
DK = D // P          # 8   D k-tiles
NT = T // 512        # 4   512-token tiles
HC = H // 2          # 8   local heads per core
DQ = HC * HD         # 512 local qkv width
NOT = DQ // P        # 4   local qkv feature tiles (head pairs)
FFT = DFF // P       # 32  DFF tiles
TOWN = T // 2        # 1024 own tokens after ReduceScatter
NTO = TOWN // 512    # 2
EPS = 1e-5
SCALE = 1.0 / 8.0    # 1/sqrt(HD)


def build_program(debug=False, sim_mode=False):
    nc = bacc.Bacc("TRN2", target_bir_lowering=False, debug=False)

    # ---- DRAM I/O ----
    xT = nc.dram_tensor("xT", [D, T], BF16, kind="ExternalInput")
    xTo = nc.dram_tensor("xTo", [D, TOWN], FP32, kind="ExternalInput")
    wqk = nc.dram_tensor("wqk", [2, NOT, DK, P, P], BF16, kind="ExternalInput")
    wv = nc.dram_tensor("wv", [DK, P, DQ], BF16, kind="ExternalInput")
    wo = nc.dram_tensor("wo", [DQ, D], BF16, kind="ExternalInput")
    w1 = nc.dram_tensor("w1", [FFT, DK, P, P], BF16, kind="ExternalInput")
    w2 = nc.dram_tensor("w2", [DK, FFT, P, P], BF16, kind="ExternalInput")
    cqk = nc.dram_tensor("cqk", [P, 2 * NOT], FP32, kind="ExternalInput")
    cvb = nc.dram_tensor("cvb", [P, DQ], BF16, kind="ExternalInput")
    bo = nc.dram_tensor("bo", [P, DK], FP32, kind="ExternalInput")
    c1 = nc.dram_tensor("c1", [P, FFT], FP32, kind="ExternalInput")
    b2 = nc.dram_tensor("b2", [P, DK], FP32, kind="ExternalInput")
    masks = nc.dram_tensor("masks", [P, 896], BF16, kind="ExternalInput")
    out = nc.dram_tensor("out", [DK, P, TOWN], FP32, kind="ExternalOutput")

    xT_r = xT.rearrange("(k p) t -> p k t", p=P)
    xTo_r = xTo.rearrange("(k p) t -> p k t", p=P)
    wo_r = wo.rearrange("(pt p) o -> p pt o", p=P)
    out_r = out.rearrange("k p t -> p k t")

    with tile.TileContext(nc) as tc:
        with (
            tc.tile_pool(name="small", bufs=1) as small,
            tc.tile_pool(name="psum", bufs=1, space="PSUM") as psum,
            tc.tile_pool(name="dram", bufs=1, space="DRAM") as dram,
        ):
            # ---- persistent small tiles / resident weights ----
            ones_bf = small.tile([P, 1], BF16)
            nc.vector.memset(ones_bf, 1.0)
            ones_r = small.tile([P, 1], FP32R)
            nc.vector.memset(ones_r.bitcast(FP32), 1.0)
            cqk_sb = small.tile([P, 2 * NOT], FP32)
            nc.sync.dma_start(cqk_sb, cqk[:, :])
            cvb_sb = small.tile([P, DQ], BF16)
            nc.sync.dma_start(cvb_sb, cvb[:, :])
            bo_sb = small.tile([P, DK], FP32)
            nc.sync.dma_start(bo_sb, bo[:, :])
            c1_sb = small.tile([P, FFT], FP32)
            nc.sync.dma_start(c1_sb, c1[:, :])
            b2_sb = small.tile([P, DK], FP32)
            nc.sync.dma_start(b2_sb, b2[:, :])
            eps_sb = small.tile([1, 1], FP32)
            nc.vector.memset(eps_sb, EPS)

            # resident weights (bf16, loaded once)
            wqk_sb = small.tile([P, 2, NOT, DK, P], BF16)
            for proj in range(2):
                nc.gpsimd.dma_start(
                    wqk_sb[:, proj], wqk[proj].rearrange("b k p m -> p b k m"))
            wv_sb = small.tile([P, DK, DQ], BF16)
            nc.gpsimd.dma_start(wv_sb, wv.rearrange("k p m -> p k m"))
            wo_sb = small.tile([P, NOT, D], BF16)
            nc.gpsimd.dma_start(wo_sb, wo_r)

            # persistent activations
            x2_sb = small.tile([P, DK, TOWN], FP32R)    # residual stream
            rsb2_sb = small.tile([P, TOWN], BF16)

            # DRAM scratch for the pairwise ReduceScatter (bf16)
            rs_in = [dram.tile([2, DK, P, 512], BF16, name=f"rsin{i}", tag=f"rsin{i}")
                     for i in range(2)]
            rs_out = [dram.tile([DK, P, 512], BF16, name=f"rsout{i}", tag=f"rsout{i}")
                      for i in range(2)]

            # ==== phases A-E: software-pipelined over 512-token tiles ====
            # iter tt: A/B(tt) stats+projections; C/D(tt-1) attention+Wo;
            # ReduceScatter + x2 + LN2 for token-half (tt-3).
            with (
                tc.tile_pool(name="kv", bufs=1) as kvp,
                tc.tile_pool(name="abc", bufs=1) as abc,
            ):
                kT_sb = kvp.tile([P, NOT, T], BF16)     # [64*(h%2)+d, h//2, t]
                v_sb = kvp.tile([P, HC, T // P, HD + 1], BF16)
                mask_sb = kvp.tile([P, 896], BF16)      # extended causal mask
                nc.sync.dma_start(mask_sb, masks[:, :])
                for h in range(HC):
                    nc.vector.memset(v_sb[:, h, :, HD:HD + 1], 1.0)

                qcur_t = [None] * NT
                for tt in range(NT + 1):
                    if tt < NT:
                        ts5 = slice(tt * 512, (tt + 1) * 512)
                        # ---- A: load x tile, LN1 stats, prescale ----
                        xt_t = abc.tile([P, DK, 512], BF16, tag="xt", bufs=2)
                        nc.sync.dma_start(xt_t, xT_r[:, :, ts5])
                        s_ps = psum.tile([1, 512], FP32, tag="st", bufs=2)
                        q_ps = psum.tile([1, 512], FP32, tag="st", bufs=2)
                        for kt in range(DK):
                            xsq = abc.tile([P, 512], BF16, tag="xsq", bufs=2)
                            nc.vector.tensor_mul(xsq, xt_t[:, kt, :], xt_t[:, kt, :])
                            nc.tensor.matmul(s_ps, ones_bf, xt_t[:, kt, :],
                                             start=(kt == 0), stop=(kt == DK - 1))
                            nc.tensor.matmul(q_ps, ones_bf, xsq,
                                             start=(kt == 0), stop=(kt == DK - 1))
                        # rows: mu, var; rstd = exp(-0.5*ln(var+eps))
                        mu = abc.tile([1, 512], FP32, tag="murow", bufs=2)
                        va = abc.tile([1, 512], FP32, tag="varow", bufs=2)
                        nc.vector.tensor_scalar(mu, s_ps, 1.0 / D, None, ALU.mult)
                        nc.vector.tensor_scalar(va, q_ps, 1.0 / D, None, ALU.mult)
                        nc.gpsimd.tensor_mul(mu, mu, mu)
                        nc.gpsimd.tensor_sub(va, va, mu)
                        lnv = abc.tile([1, 512], FP32, tag="lnrow", bufs=2)
                        nc.scalar.activation(lnv, va, AF.Ln, bias=eps_sb)
                        rstd_row = abc.tile([1, 512], BF16, tag="rsrow", bufs=2)
                        nc.scalar.activation(rstd_row, lnv, AF.Exp, scale=-0.5)
                        rsb = abc.tile([P, 512], BF16, tag="rsb", bufs=2)
                        nc.gpsimd.partition_broadcast(rsb, rstd_row)
                        # xh = x * rstd  (bf16, the matmul operand everywhere)
                        xh_t = abc.tile([P, DK, 512], BF16, tag="xh", bufs=2)
                        for kt in range(DK):
                            nc.vector.tensor_mul(xh_t[:, kt, :], xt_t[:, kt, :], rsb)

                        # ---- B: q/k projections (feature-major out) ----
                        qcur = abc.tile([P, NOT, 512], BF16, tag="qcur", bufs=2)
                        qcur_t[tt] = qcur
                        for proj in range(2):  # 0=q, 1=k
                            for ot in range(NOT):
                                pp = psum.tile([P, 512], FP32, tag="mm", bufs=2)
                                for kt in range(DK):
                                    nc.tensor.matmul(
                                        pp, wqk_sb[:, proj, ot, kt, :],
                                        xh_t[:, kt, :],
                                        start=(kt == 0), stop=(kt == DK - 1))
                                dest = (qcur[:, ot, :] if proj == 0
                                        else kT_sb[:, ot, ts5])
                                nc.vector.tensor_scalar(
                                    dest, pp,
                                    cqk_sb[:, proj * NOT + ot:proj * NOT + ot + 1],
                                    None, ALU.add)
                        # ---- B: v projection (token-major out) ----
                        for st in range(4):
                            pp = psum.tile([P, 512], FP32, tag="mm", bufs=2)
                            for kt in range(DK):
                                nc.tensor.matmul(
                                    pp, xh_t[:, kt, st * P:(st + 1) * P],
                                    wv_sb[:, kt, :], start=(kt == 0),
                                    stop=(kt == DK - 1))
                            nc.vector.tensor_tensor(
                                v_sb[:, :, tt * 4 + st, 0:HD],
                                pp.rearrange("p (h e) -> p h e", h=HC),
                                cvb_sb.rearrange("p (h e) -> p h e", h=HC), ALU.add)

                    if tt >= 1:
                        # ---- C: attention for q-tile qt = tt-1 ----
                        qt = tt - 1
                        qv = qcur_t[qt]
                        nkt = 4 * qt + 4
                        ysb = abc.tile([P, NOT, 512], BF16, tag="ysb", bufs=1)
                        for pt in range(NOT):
                            y_ps = [psum.tile([HD + 1, 512], FP32, name=f"yps{hb}",
                                              tag="y", bufs=2) for hb in range(2)]
                            for kt in range(nkt):
                                jband = kt - 4 * qt
                                # queries below 128*jband cannot attend to this
                                # key block: restrict all work to [q0, 512).
                                q0 = P * jband if jband > 0 else 0
                                qs = slice(q0, 512)
                                pexp = []
                                for hb in range(2):
                                    hsl = slice(hb * HD, (hb + 1) * HD)
                                    s_ps2 = psum.tile([P, 512], FP32, tag="s", bufs=2)
                                    nc.tensor.matmul(
                                        s_ps2[:, qs],
                                        kT_sb[hsl, pt, kt * P:(kt + 1) * P],
                                        qv[hsl, pt, qs], start=True, stop=True)
                                    pe = abc.tile([P, 512], BF16, tag="pexp", bufs=3)
                                    nc.scalar.activation(pe[:, qs], s_ps2[:, qs],
                                                         AF.Exp, scale=SCALE)
                                    if jband >= 0:
                                        moff = 384 - P * jband
                                        nc.vector.tensor_mul(
                                            pe[:, qs], pe[:, qs],
                                            mask_sb[:, moff + q0:moff + 512])
                                    pexp.append(pe)
                                for hb in range(2):
                                    nc.tensor.matmul(
                                        y_ps[hb][:, qs],
                                        v_sb[:, 2 * pt + hb, kt, :], pexp[hb][:, qs],
                                        start=(kt == 0), stop=(kt == nkt - 1))
                            for hb in range(2):
                                # denominator: reciprocal on its own lane, DMA
                                # the row down to lane 0, gpsimd-broadcast.
                                den = abc.tile([HD + 1, 512], FP32, tag="den", bufs=2)
                                nc.vector.reciprocal(den[HD:HD + 1, :],
                                                     y_ps[hb][HD:HD + 1, :])
                                rec = abc.tile([1, 512], FP32, tag="rec", bufs=2)
                                nc.gpsimd.dma_start(rec, den[HD:HD + 1, :])
                                rb = abc.tile([HD, 512], FP32, tag="rb", bufs=2)
                                nc.gpsimd.partition_broadcast(rb, rec)
                                if hb == 0:
                                    nc.vector.tensor_mul(ysb[0:HD, pt, :],
                                                         y_ps[hb][0:HD, :], rb)
                                else:
                                    yst = abc.tile([HD, 512], BF16, tag="yst", bufs=2)
                                    nc.vector.tensor_mul(yst, y_ps[hb][0:HD, :], rb)
                                    nc.gpsimd.dma_start(ysb[HD:2 * HD, pt, :], yst)
                        # ---- D: Wo partials for q-tile qt ----
                        for ot in range(DK):
                            pp = psum.tile([P, 512], FP32, tag="mm", bufs=2)
                            for pt in range(NOT):
                                nc.tensor.matmul(
                                    pp, wo_sb[:, pt, ot * P:(ot + 1) * P],
                                    ysb[:, pt, :],
                                    start=(pt == 0), stop=(pt == NOT - 1))
                            ast = abc.tile([P, 512], BF16, tag="ast", bufs=2)
                            nc.vector.tensor_copy(ast, pp)
                            nc.sync.dma_start(rs_in[qt % 2][qt // 2, ot], ast)
                        if qt >= 2:
                            # ---- ReduceScatter + x2 + LN2 for half i ----
                            i = qt - 2
                            io5 = slice(i * 512, (i + 1) * 512)
                            if sim_mode:
                                nc.sync.dma_start(rs_out[i][:, :, :], rs_in[i][0])
                            else:
                                nc.gpsimd.collective_compute(
                                    "ReduceScatter", ALU.add,
                                    replica_groups=[[0, 1], [2, 3], [4, 5], [6, 7]],
                                    ins=[rs_in[i].opt()], outs=[rs_out[i].opt()])
                            s2_ps = psum.tile([1, 512], FP32, tag="st", bufs=2)
                            q2_ps = psum.tile([1, 512], FP32, tag="st", bufs=2)
                            for kt in range(DK):
                                att = abc.tile([P, 512], BF16, tag="att", bufs=2)
                                nc.sync.dma_start(att, rs_out[i][kt])
                                xo_t = abc.tile([P, 512], FP32, tag="xo", bufs=2)
                                nc.sync.dma_start(xo_t, xTo_r[:, kt, io5])
                                nc.vector.scalar_tensor_tensor(
                                    x2_sb[:, kt, io5], att, bo_sb[:, kt:kt + 1],
                                    xo_t, ALU.add, ALU.add)
                                xsq2 = abc.tile([P, 512], FP32R, tag="xsq2", bufs=2)
                                nc.vector.tensor_mul(xsq2, x2_sb[:, kt, io5],
                                                     x2_sb[:, kt, io5])
                                nc.tensor.matmul(s2_ps, ones_r, x2_sb[:, kt, io5],
                                                 start=(kt == 0), stop=(kt == DK - 1))
                                nc.tensor.matmul(q2_ps, ones_r, xsq2,
                                                 start=(kt == 0), stop=(kt == DK - 1))
                            mu2 = abc.tile([1, 512], FP32, tag="murow", bufs=2)
                            va2 = abc.tile([1, 512], FP32, tag="varow", bufs=2)
                            nc.vector.tensor_scalar(mu2, s2_ps, 1.0 / D, None, ALU.mult)
                            nc.vector.tensor_scalar(va2, q2_ps, 1.0 / D, None, ALU.mult)
                            nc.gpsimd.tensor_mul(mu2, mu2, mu2)
                            nc.gpsimd.tensor_sub(va2, va2, mu2)
                            lnv2 = abc.tile([1, 512], FP32, tag="lnrow", bufs=2)
                            nc.scalar.activation(lnv2, va2, AF.Ln, bias=eps_sb)
                            rstd2_row = abc.tile([1, 512], BF16, tag="rsrow", bufs=2)
                            nc.scalar.activation(rstd2_row, lnv2, AF.Exp, scale=-0.5)
                            nc.gpsimd.partition_broadcast(rsb2_sb[:, io5], rstd2_row)

            # =========== phases F/G: MLP over 512-token tiles ==========
            with tc.tile_pool(name="fg", bufs=1) as fg:
                for tt in range(NTO):
                    ts5 = slice(tt * 512, (tt + 1) * 512)
                    xh2_t = fg.tile([P, DK, 512], BF16, tag="xh2", bufs=1)
                    for kt in range(DK):
                        nc.vector.tensor_mul(xh2_t[:, kt, :], x2_sb[:, kt, ts5],
                                             rsb2_sb[:, ts5])
                    m_sb = fg.tile([P, FFT, 512], BF16, tag="m", bufs=1)
                    for fft in range(FFT):
                        w1b = fg.tile([P, DK, P], BF16, tag="w1b", bufs=3)
                        nc.gpsimd.dma_start(w1b, w1[fft].rearrange("k p m -> p k m"))
                        pp = psum.tile([P, 512], FP32, tag="mm", bufs=2)
                        for kt in range(DK):
                            nc.tensor.matmul(pp, w1b[:, kt, :], xh2_t[:, kt, :],
                                             start=(kt == 0),
                                             stop=(kt == DK - 1))
                        nc.scalar.activation(m_sb[:, fft, :], pp, AF.Gelu,
                                             bias=c1_sb[:, fft:fft + 1])
                    for ot in range(DK):
                        pp = psum.tile([P, 512], FP32, tag="mm", bufs=2)
                        for half in range(2):
                            w2b = fg.tile([P, FFT // 2, P], BF16, tag="w2b", bufs=2)
                            nc.gpsimd.dma_start(
                                w2b, w2[ot, half * (FFT // 2):(half + 1) * (FFT // 2)]
                                .rearrange("k p m -> p k m"))
                            for kk in range(FFT // 2):
                                kt = half * (FFT // 2) + kk
                                nc.tensor.matmul(pp, w2b[:, kk, :], m_sb[:, kt, :],
                                                 start=(kt == 0), stop=(kt == FFT - 1))
                        ost = fg.tile([P, 512], FP32, tag="ost", bufs=2)
                        nc.vector.scalar_tensor_tensor(
                            ost, pp, b2_sb[:, ot:ot + 1], x2_sb[:, ot, ts5],
                            ALU.add, ALU.add)
                        nc.sync.dma_start(out_r[:, ot, ts5], ost)

    nc.compile()
    return nc


_NC_CACHE = None


def _get_nc():
    global _NC_CACHE
    if _NC_CACHE is None:
        _NC_CACHE = build_program(debug=bool(int(os.environ.get("KERNEL_DEBUG", "0"))))
    return _NC_CACHE


def prep_in_maps(x, ln1_g, ln1_b, ln2_g, ln2_b, Wq, bq, Wk, bk, Wv, bv,
                 Wo, bo, W1, b1, W2, b2):
    from ml_dtypes import bfloat16
    f32 = np.float32
    x = np.asarray(x, f32)
    ln1_g, ln1_b = np.asarray(ln1_g, f32), np.asarray(ln1_b, f32)
    ln2_g, ln2_b = np.asarray(ln2_g, f32), np.asarray(ln2_b, f32)
    Wq, Wk, Wv, Wo = (np.asarray(a, f32) for a in (Wq, Wk, Wv, Wo))
    W1, W2 = np.asarray(W1, f32), np.asarray(W2, f32)
    bq, bk, bv, bo_, b1, b2_ = (np.asarray(a, f32) for a in (bq, bk, bv, bo, b1, b2))

    # fold LN gain AND the mean subtraction (a rank-1 correction) into W:
    # (x - mu) * g @ W  =  x @ (g*W - colsum(g*W)/D)
    Wqg = ln1_g[:, None] * Wq
    Wkg = ln1_g[:, None] * Wk
    Wvg = ln1_g[:, None] * Wv
    Wqg = Wqg - Wqg.sum(0, keepdims=True) / D
    Wkg = Wkg - Wkg.sum(0, keepdims=True) / D
    Wvg = Wvg - Wvg.sum(0, keepdims=True) / D
    cq_full = ln1_b @ Wq + bq
    ck_full = ln1_b @ Wk + bk
    cv_full = ln1_b @ Wv + bv
    W1g = ln2_g[:, None] * W1
    W1g = W1g - W1g.sum(0, keepdims=True) / D
    c1_full = ln2_b @ W1 + b1

    w1_t = np.ascontiguousarray(
        W1g.reshape(DK, P, FFT, P).transpose(2, 0, 1, 3)).astype(bfloat16)
    w2_t = np.ascontiguousarray(
        W2.reshape(FFT, P, DK, P).transpose(2, 0, 1, 3)).astype(bfloat16)
    c1_t = np.ascontiguousarray(c1_full.reshape(FFT, P).T)      # [P,FFT]
    b2_t = np.ascontiguousarray(b2_.reshape(DK, P).T)           # [P,DK]
    bo_t = np.ascontiguousarray(bo_.reshape(DK, P).T)           # [P,DK]

    kk = np.arange(P)[:, None]
    cc = np.arange(896)[None, :]
    mk = (kk + 384 <= cc).astype(bfloat16)

    in_maps = []
    for c in range(8):
        b_idx, hh = c // 2, c % 2
        sl = slice(DQ * hh, DQ * hh + DQ)
        xT_c = np.ascontiguousarray(x[b_idx].T)
        wq_c, wk_c = Wqg[:, sl], Wkg[:, sl]
        wqk_t = np.stack([
            np.ascontiguousarray(w.reshape(DK, P, NOT, P).transpose(2, 0, 1, 3))
            for w in (wq_c, wk_c)]).astype(bfloat16)             # [2,NOT,DK,P,P]
        cq_t = cq_full[sl].reshape(NOT, P).T                     # [P,NOT]
        ck_t = ck_full[sl].reshape(NOT, P).T
        in_maps.append({
            "xT": xT_c.astype(bfloat16),
            "xTo": np.ascontiguousarray(xT_c[:, hh * TOWN:(hh + 1) * TOWN]),
            "wqk": wqk_t,
            "wv": np.ascontiguousarray(Wvg[:, sl].reshape(DK, P, DQ)).astype(bfloat16),
            "wo": np.ascontiguousarray(Wo[sl, :]).astype(bfloat16),
            "w1": w1_t,
            "w2": w2_t,
            "cqk": np.ascontiguousarray(np.concatenate([cq_t, ck_t], axis=1)),
            "cvb": np.broadcast_to(cv_full[sl][None, :], (P, DQ)).astype(bfloat16),
            "bo": bo_t,
            "c1": c1_t,
            "b2": b2_t,
            "masks": mk,
        })
    return in_maps


def assemble_output(results):
    out = np.empty((B, T, D), np.float32)
    for c in range(8):
        b_idx, hh = c // 2, c % 2
        o = results[c]["out"].reshape(D, TOWN)
        out[b_idx, hh * TOWN:(hh + 1) * TOWN, :] = o.T
    return out


def kernel(**inputs):
    nc = _get_nc()
    in_maps = prep_in_maps(**inputs)
    res = run_bass_kernel_spmd(nc, in_maps, list(range(8)))
    return assemble_output(res.results)
